# revision 8
# baseline (speedup 1.0000x reference)
"""MPN-COV pooling + projection kernel for 8 Trainium2 NeuronCores.

Problem: nn_PillTeacher_48661979464182
  feat [64, 256, 14, 14] -> per-sample covariance + 3 Newton-Schulz sqrt
  iterations -> L2-normalize -> project with W_proj [512, 65536] -> BN -> L2.

Sharding:
  - Pooling phase: pure data parallel, 8 samples per core.
  - Projection: k-shard of W_proj (each core holds an 8192-wide slice of the
    contraction dim). AllToAll exchanges the normalized pooled matrices so
    every core gets its k-slice of all 64 samples; partial embeddings are
    summed with ReduceScatter back to the owning core of each sample.

Key tricks:
  - Every matrix in the Newton-Schulz iteration is a polynomial of the
    (symmetric) covariance -> symmetric -> matmul lhsT operands read the
    row-major tiles directly (no transposes on device; feat pre-transposed
    on host).
  - The final L2 normalization is invariant to any positive per-sample
    scale, so 1/M, 1/trace, sqrt(trY) and the 0.5 of the last NS Y-update
    all drop out.
  - fp32r (4x-rate fp32 matmul mode) for all matmuls.
  - BN + bias folded into a host-computed scale/shift.

Workarounds for this walrus build:
  - <=1 semaphore wait per instruction (_split_excess_waits post-pass).
  - no matmul with rhs free size 1 (scalar reductions go through [1, 256]
    row-sums + a free-axis reduce; scalar broadcasts use [1, 2] operands).
  - no tensor_tensor_reduce (mask-mult + tensor_reduce / activation instead).
"""
import sys
import numpy as np

sys.path.insert(0, "/opt/trn_rl_repo")

import concourse.bass as bass
import concourse.mybir as mybir
import concourse.tile as tile
import bass_rust
from concourse.bass_utils import run_bass_kernel_spmd

dt = mybir.dt

N_CORES = 8
B, C, H, W_SP = 64, 256, 14, 14
M = H * W_SP           # 196
E = 512
K = C * C              # 65536
BL = B // N_CORES      # 8 samples per core
KL = K // N_CORES      # 8192 contraction slice per core
BN_EPS = 1e-5

_cache = {}


def _split_excess_waits(nc, max_waits=1):
    """walrus in this env rejects >1 semaphore wait per instruction; hoist
    excess waits onto preceding NoOps on the same engine."""
    for fn in nc.m.functions:
        for bb in fn.blocks:
            new_insts = []
            for inst in bb.instructions:
                si = inst.sync_info
                if si is not None and si.on_wait and len(si.on_wait) > max_waits:
                    waits = list(si.on_wait)
                    chunks = [waits[i:i + max_waits]
                              for i in range(0, len(waits), max_waits)]
                    for chunk in chunks[:-1]:
                        nop = mybir.InstNoOp(
                            name=nc.get_next_instruction_name(), ins=[], outs=[],
                            engine=inst.engine)
                        nop.sync_info = bass_rust.SyncInfo(on_wait=chunk,
                                                           on_update=[])
                        new_insts.append(nop)
                    si.on_wait = chunks[-1]
                new_insts.append(inst)
            bb.instructions = new_insts


def _build(stage=5):
    """stage: 1=Y0 dump, 2=F dump, 3=a2a_out dump, 4=emb partial dump,
    5=full kernel."""
    f32, f32r = dt.float32, dt.float32r
    nc = bass.Bass("TRN2", target_bir_lowering=False, debug=False,
                   num_devices=N_CORES)

    featT = nc.dram_tensor("featT", [BL, M, C], f32r, kind="ExternalInput")
    onesc = nc.dram_tensor("onesc", [128, 1], f32r, kind="ExternalInput")
    onesr = nc.dram_tensor("onesr", [1, 128], f32r, kind="ExternalInput")
    ident = nc.dram_tensor("ident", [2, 128, C], f32, kind="ExternalInput")
    if stage >= 4:
        wT = nc.dram_tensor("wT", [KL, E], f32r, kind="ExternalInput")
    if stage >= 5:
        bnsc = nc.dram_tensor("bnsc", [BL, E], f32, kind="ExternalInput")
        bnsh = nc.dram_tensor("bnsh", [BL, E], f32, kind="ExternalInput")
        out = nc.dram_tensor("out", [BL, E], f32, kind="ExternalOutput")
    elif stage <= 2:
        out = nc.dram_tensor("dbg", [2, 128, C], f32, kind="ExternalOutput")
    elif stage == 3:
        out = nc.dram_tensor("dbg", [128, 4096], f32, kind="ExternalOutput")
    else:
        out = nc.dram_tensor("dbg", [64, E], f32, kind="ExternalOutput")

    rg = [list(range(N_CORES))]
    AluOp = mybir.AluOpType
    NCH = KL // 128        # 64 k-chunks for the projection

    lp = nc.allow_low_precision(reason="f32r intermediates carry fp32 bits")
    lp.__enter__()
    with tile.TileContext(nc) as tc:
        with (
            tc.tile_pool(name="consts", bufs=1) as cpool,
            tc.tile_pool(name="wbuf", bufs=1) as wpool,
            tc.tile_pool(name="big", bufs=1) as bigpool,
            tc.tile_pool(name="work", bufs=2) as pool,
            tc.tile_pool(name="mats", bufs=2) as mats,
            tc.tile_pool(name="pss", bufs=1, space="PSUM") as pss,
            tc.tile_pool(name="psg", bufs=2, space="PSUM") as psg,
            tc.tile_pool(name="pse", bufs=1, space="PSUM") as pse,
            tc.tile_pool(name="dram", bufs=1, space="DRAM") as dram,
        ):
            # ---------- constants ----------
            ones_t = cpool.tile([128, 1], f32r, name="ones_t")
            nc.sync.dma_start(ones_t[:], onesc[:])
            onesr_t = cpool.tile([1, 128], f32r, name="onesr_t")
            nc.sync.dma_start(onesr_t[:], onesr[:])
            ident_t = [cpool.tile([128, C], f32, name=f"id{r}") for r in range(2)]
            threeI_t = [cpool.tile([128, C], f32, name=f"id3{r}") for r in range(2)]
            for r in range(2):
                nc.sync.dma_start(ident_t[r][:], ident[r, :, :])
                nc.scalar.mul(threeI_t[r][:], ident_t[r][:], 3.0)
            if stage >= 5:
                bnsc_t = cpool.tile([BL, E], f32, name="bnsc_t")
                bnsh_t = cpool.tile([BL, E], f32, name="bnsh_t")
                nc.sync.dma_start(bnsc_t[:], bnsc[:])
                nc.sync.dma_start(bnsh_t[:], bnsh[:])

            # ---------- W prefetch: [KL, E] -> SBUF [128, NCH*E] ----------
            if stage >= 4:
                Wq = wpool.tile([128, NCH * E], f32r, name="Wq")
                wT_v = wT.rearrange("(g p) e -> p g e", p=128)   # [128,64,512]
                for g in range(8):
                    nc.sync.dma_start(
                        Wq[:, g * 8 * E:(g + 1) * 8 * E]
                            .rearrange("p (c e) -> p c e", c=8),
                        wT_v[:, 8 * g:8 * (g + 1), :])

            # ---------- DRAM staging for collectives ----------
            if stage >= 3:
                # a2a flat layout: [j(8), h(2), p(128), b_l(8), i0(32)]
                a2a_in = dram.tile([128, 4096], f32r, name="a2a_in")
                a2a_out = dram.tile([128, 4096], f32r, name="a2a_out")
                a2a_in_v = a2a_in.flatten().rearrange(
                    "(j h p b i) -> h b p j i", j=8, h=2, p=128, b=BL, i=32)
            if stage >= 5:
                rs_in = dram.tile([B, E], f32, name="rs_in")
                rs_out = dram.tile([BL, E], f32, name="rs_out")

            def mm4(out_pair, A_pair, B_pair):
                """out = A @ B for symmetric A; pairs of [128, 256] row tiles."""
                for r in range(2):
                    for kc in range(2):
                        nc.tensor.matmul(
                            out_pair[r][:],
                            A_pair[kc][:, 128 * r:128 * (r + 1)],
                            B_pair[kc][:],
                            start=(kc == 0), stop=(kc == 1))

            def rowsum2(ps_tile, rhs0, rhs1):
                """ps_tile[1, 256] = colsums of rhs0 + colsums of rhs1."""
                nc.tensor.matmul(ps_tile[:], ones_t[0:128, :], rhs0[:],
                                 start=True, stop=False)
                nc.tensor.matmul(ps_tile[:], ones_t[0:128, :], rhs1[:],
                                 start=False, stop=True)

            def scalar_bcast(val_sb, tag):
                """[1,1] f32r scalar -> [128,1] f32 SBUF (via N=2 matmul)."""
                v2 = pool.tile([1, 2], f32r, name=f"v2{tag}", tag=f"v2{tag}")
                nc.vector.tensor_copy(v2[:, 0:1], val_sb[:])
                nc.vector.tensor_copy(v2[:, 1:2], val_sb[:])
                b_ps = pss.tile([128, 2], f32, name=f"bps{tag}", tag="sm2")
                nc.tensor.matmul(b_ps[:], onesr_t[:], v2[:],
                                 start=True, stop=True)
                b_sb = pool.tile([128, 1], f32, name=f"bsb{tag}", tag=f"bsb{tag}")
                nc.vector.tensor_copy(b_sb[:], b_ps[:, 0:1])
                return b_sb

            # ---------- pooling phase: BL samples ----------
            nsamp = 1 if stage <= 2 else BL
            for b in range(nsamp):
                B0 = pool.tile([128, C], f32r, name="B0", tag="B0")
                B1 = pool.tile([M - 128, C], f32r, name="B1", tag="B1")
                nc.sync.dma_start(B0[:], featT[b, 0:128, :])
                nc.sync.dma_start(B1[:], featT[b, 128:M, :])

                # column sums -> [1, 256]
                srow_ps = pss.tile([1, C], f32, name="srow", tag="sm1")
                nc.tensor.matmul(srow_ps[:], ones_t[0:128, :], B0[:],
                                 start=True, stop=False)
                nc.tensor.matmul(srow_ps[:], ones_t[0:M - 128, :], B1[:],
                                 start=False, stop=True)
                s_sb = pool.tile([1, C], f32r, name="s_sb", tag="s_sb")
                t_sb = pool.tile([1, C], f32r, name="t_sb", tag="t_sb")
                nc.scalar.copy(s_sb[:], srow_ps[:])
                nc.scalar.mul(t_sb[:], srow_ps[:], -1.0 / M)

                # G = A^T A - M xbar xbar^T   (PSUM pair [128, 256])
                G_ps = [psg.tile([128, C], f32, name=f"G{r}", tag=f"Yp{r}")
                        for r in range(2)]
                for r in range(2):
                    nc.tensor.matmul(G_ps[r][:], B0[:, 128 * r:128 * (r + 1)],
                                     B0[:], start=True, stop=False)
                    nc.tensor.matmul(G_ps[r][:], B1[:, 128 * r:128 * (r + 1)],
                                     B1[:], start=False, stop=False)
                    nc.tensor.matmul(G_ps[r][:], t_sb[:, 128 * r:128 * (r + 1)],
                                     s_sb[:], start=False, stop=True)

                # trace: scr_r = G_r * I_r (f32r) -> colsum row -> free reduce
                scr = [pool.tile([128, C], f32r, name=f"scr{r}", tag=f"scr{r}")
                       for r in range(2)]
                for r in range(2):
                    nc.vector.tensor_tensor(scr[r][:], G_ps[r][:],
                                            ident_t[r][:], AluOp.mult)
                trrow_ps = pss.tile([1, C], f32, name="trrow", tag="sm1")
                rowsum2(trrow_ps, scr[0], scr[1])
                tr_sb = pool.tile([1, 1], f32, name="tr_sb", tag="tr_sb")
                nc.vector.tensor_reduce(out=tr_sb[:], in_=trrow_ps[:],
                                        axis=mybir.AxisListType.X,
                                        op=AluOp.add)
                inv_sb = pool.tile([1, 1], f32r, name="inv_sb", tag="inv")
                nc.vector.reciprocal(inv_sb[:], tr_sb[:])
                invb = scalar_bcast(inv_sb, "i")

                # Y0 = G / trG
                Y0 = [mats.tile([128, C], f32r, name=f"Y0{r}", tag=f"Y0{r}")
                      for r in range(2)]
                for r in range(2):
                    nc.vector.tensor_scalar_mul(Y0[r][:], G_ps[r][:], invb[:])

                if stage == 1:
                    for r in range(2):
                        nc.sync.dma_start(out[r, :, :], Y0[r][:].bitcast(f32))
                    break

                # ---- NS iter 1 (Z0=I): T1 = 3I - Y0; Y1 = .5 Y0 T1; Z1 = .5 T1
                T1 = [mats.tile([128, C], f32r, name=f"T1{r}", tag=f"T{r}")
                      for r in range(2)]
                for r in range(2):
                    nc.vector.scalar_tensor_tensor(
                        out=T1[r][:], in0=Y0[r][:], scalar=-1.0,
                        in1=threeI_t[r][:], op0=AluOp.mult, op1=AluOp.add)
                Yp = [psg.tile([128, C], f32, name=f"Yp1{r}", tag=f"Yp{r}")
                      for r in range(2)]
                mm4(Yp, Y0, T1)
                Y1 = [mats.tile([128, C], f32r, name=f"Y1{r}", tag=f"Y1{r}")
                      for r in range(2)]
                Z1 = [mats.tile([128, C], f32r, name=f"Z1{r}", tag=f"Z1{r}")
                      for r in range(2)]
                for r in range(2):
                    nc.scalar.mul(Y1[r][:], Yp[r][:], 0.5)
                    nc.scalar.mul(Z1[r][:], T1[r][:], 0.5)

                # ---- NS iter 2
                Pp = [psg.tile([128, C], f32, name=f"Pp{r}", tag=f"Yp{r}")
                      for r in range(2)]
                mm4(Pp, Z1, Y1)
                T2 = [mats.tile([128, C], f32r, name=f"T2{r}", tag=f"T{r}")
                      for r in range(2)]
                for r in range(2):
                    nc.vector.scalar_tensor_tensor(
                        out=T2[r][:], in0=Pp[r][:], scalar=-1.0,
                        in1=threeI_t[r][:], op0=AluOp.mult, op1=AluOp.add)
                Yp2 = [psg.tile([128, C], f32, name=f"Yp2{r}", tag=f"Yp{r}")
                       for r in range(2)]
                mm4(Yp2, Y1, T2)
                Y2 = [mats.tile([128, C], f32r, name=f"Y2{r}", tag=f"Y0{r}")
                      for r in range(2)]
                for r in range(2):
                    nc.scalar.mul(Y2[r][:], Yp2[r][:], 0.5)
                Zp = [psg.tile([128, C], f32, name=f"Zp{r}", tag=f"Yp{r}")
                      for r in range(2)]
                mm4(Zp, T2, Z1)
                Z2 = [mats.tile([128, C], f32r, name=f"Z2{r}", tag=f"Z1{r}")
                      for r in range(2)]
                for r in range(2):
                    nc.scalar.mul(Z2[r][:], Zp[r][:], 0.5)

                # ---- NS iter 3 (Z dead): Y3 = Y2 (3I - Z2 Y2), unscaled
                Pp3 = [psg.tile([128, C], f32, name=f"Pp3{r}", tag=f"Yp{r}")
                       for r in range(2)]
                mm4(Pp3, Z2, Y2)
                T3 = [mats.tile([128, C], f32r, name=f"T3{r}", tag=f"T{r}")
                      for r in range(2)]
                for r in range(2):
                    nc.vector.scalar_tensor_tensor(
                        out=T3[r][:], in0=Pp3[r][:], scalar=-1.0,
                        in1=threeI_t[r][:], op0=AluOp.mult, op1=AluOp.add)
                Y3p = [psg.tile([128, C], f32, name=f"Y3p{r}", tag=f"Yp{r}")
                       for r in range(2)]
                mm4(Y3p, Y2, T3)

                # ---- flat-normalize: F = Y3 / ||Y3||_F
                sq = [pool.tile([128, C], f32r, name=f"sq{r}", tag=f"scr{r}")
                      for r in range(2)]
                for r in range(2):
                    nc.scalar.activation(sq[r][:], Y3p[r][:],
                                         mybir.ActivationFunctionType.Square)
                ssqrow_ps = pss.tile([1, C], f32, name="ssqrow", tag="sm1")
                rowsum2(ssqrow_ps, sq[0], sq[1])
                ssq_sb = pool.tile([1, 1], f32, name="ssq_sb", tag="tr_sb")
                nc.vector.tensor_reduce(out=ssq_sb[:], in_=ssqrow_ps[:],
                                        axis=mybir.AxisListType.X,
                                        op=AluOp.add)
                sqr_sb = pool.tile([1, 1], f32, name="sqr_sb", tag="sqr")
                nc.scalar.sqrt(sqr_sb[:], ssq_sb[:])
                rsq_sb = pool.tile([1, 1], f32r, name="rsq_sb", tag="inv")
                nc.vector.reciprocal(rsq_sb[:], sqr_sb[:])
                rsqb = scalar_bcast(rsq_sb, "r")
                F = [mats.tile([128, C], f32r, name=f"F{r}", tag=f"F{r}")
                     for r in range(2)]
                for r in range(2):
                    nc.vector.tensor_scalar_mul(F[r][:], Y3p[r][:], rsqb[:])

                if stage == 2:
                    for r in range(2):
                        nc.sync.dma_start(out[r, :, :], F[r][:].bitcast(f32))
                    break

                # ---- staging: dest j gets F[:, 32j:32j+32] (symmetry!)
                for hh in range(2):
                    nc.sync.dma_start(
                        a2a_in_v[hh, b],                          # [p, j, i]
                        F[hh][:].rearrange("p (j i) -> p j i", j=8, i=32))

            # ---------- AllToAll ----------
            if stage >= 3:
                nc.gpsimd.collective_compute(
                    "AllToAll", AluOp.bypass, replica_groups=rg,
                    ins=[a2a_in.opt()], outs=[a2a_out.opt()])

            if stage == 3:
                tmp = bigpool.tile([128, 4096], f32, name="tmp")
                nc.sync.dma_start(tmp[:], a2a_out[:].bitcast(f32))
                nc.sync.dma_start(out[:], tmp[:])

            if stage >= 4:
                # ------- consumer: BIG [128, 4096], free = [h, s, b, i] -------
                BIG = bigpool.tile([128, 2 * 8 * BL * 32], f32r, name="BIG")
                a2a_out_v = a2a_out.flatten().rearrange(
                    "(s h p b i) -> h s p b i", s=8, h=2, p=128, b=BL, i=32)
                BIG_v = BIG[:].rearrange("p (h s b i) -> h s p b i",
                                         h=2, s=8, b=BL, i=32)
                for hh in range(2):
                    for s in range(8):
                        nc.sync.dma_start(BIG_v[hh, s], a2a_out_v[hh, s])

                # ------- projection: EMB[64, 512] -------
                EMB = pse.tile([64, E], f32, name="EMB")
                BIG_k = BIG[:].rearrange("p (h sb i) -> h i p sb",
                                         h=2, sb=64, i=32)
                for c in range(NCH):
                    i_local, hh = c // 2, c % 2
                    nc.tensor.matmul(
                        EMB[:], BIG_k[hh, i_local], Wq[:, E * c:E * (c + 1)],
                        start=(c == 0), stop=(c == NCH - 1))

                emb_sb = pool.tile([64, E], f32, name="emb_sb", tag="emb")
                nc.vector.tensor_copy(emb_sb[:], EMB[:])
                if stage == 4:
                    nc.sync.dma_start(out[:], emb_sb[:])

            if stage >= 5:
                nc.sync.dma_start(rs_in[:], emb_sb[:])

                # ------- ReduceScatter: [64, E] -> [8, E] -------
                nc.gpsimd.collective_compute(
                    "ReduceScatter", AluOp.add, replica_groups=rg,
                    ins=[rs_in.opt()], outs=[rs_out.opt()])

                # ------- BN fold + final L2 normalize -------
                e_sb = pool.tile([BL, E], f32, name="e_sb", tag="fin")
                nc.sync.dma_start(e_sb[:], rs_out[:])
                e_bn = pool.tile([BL, E], f32, name="e_bn", tag="fin2")
                nc.vector.tensor_tensor(e_bn[:], e_sb[:], bnsc_t[:], AluOp.mult)
                nc.vector.tensor_tensor(e_bn[:], e_bn[:], bnsh_t[:], AluOp.add)
                scr3 = pool.tile([BL, E], f32, name="scr3", tag="fin")
                nrm_sb = pool.tile([BL, 1], f32, name="nrm_sb", tag="nrm")
                nc.scalar.activation(
                    scr3[:], e_bn[:], mybir.ActivationFunctionType.Square,
                    accum_out=nrm_sb[:])
                nrms_sb = pool.tile([BL, 1], f32, name="nrms_sb", tag="nrms")
                nc.scalar.sqrt(nrms_sb[:], nrm_sb[:])
                rs_sb = pool.tile([BL, 1], f32, name="rs_sb", tag="nrmr")
                nc.vector.reciprocal(rs_sb[:], nrms_sb[:])
                e_fin = pool.tile([BL, E], f32, name="e_fin", tag="fin3")
                nc.vector.tensor_scalar_mul(e_fin[:], e_bn[:], rs_sb[:])
                nc.sync.dma_start(out[:], e_fin[:])

    _split_excess_waits(nc)
    return nc


def host_inputs(feat, W_proj, b_proj, bn_gamma, bn_beta, bn_mean, bn_var):
    """Build the 8 per-core input maps."""
    feat = np.ascontiguousarray(np.asarray(feat, dtype=np.float32))
    W_proj = np.asarray(W_proj, dtype=np.float32)
    featT = feat.reshape(B, C, M).transpose(0, 2, 1)          # [64, 196, 256]
    bnscale = (np.asarray(bn_gamma) /
               np.sqrt(np.asarray(bn_var) + BN_EPS)).astype(np.float32)
    bnshift = ((np.asarray(b_proj) - np.asarray(bn_mean)) * bnscale
               + np.asarray(bn_beta)).astype(np.float32)
    bnsc_rep = np.ascontiguousarray(np.broadcast_to(bnscale, (BL, E)))
    bnsh_rep = np.ascontiguousarray(np.broadcast_to(bnshift, (BL, E)))

    onesc = np.ones((128, 1), np.float32)
    onesr = np.ones((1, 128), np.float32)
    ident = np.zeros((2, 128, C), np.float32)
    ident[0, :, 0:128] = np.eye(128, dtype=np.float32)
    ident[1, :, 128:256] = np.eye(128, dtype=np.float32)

    in_maps = []
    for i in range(N_CORES):
        in_maps.append({
            "featT": np.ascontiguousarray(featT[i * BL:(i + 1) * BL]),
            "wT": np.ascontiguousarray(W_proj[:, KL * i:KL * (i + 1)].T),
            "onesc": onesc, "onesr": onesr, "ident": ident,
            "bnsc": bnsc_rep, "bnsh": bnsh_rep,
        })
    return in_maps


def kernel(feat, W_proj, b_proj, bn_gamma, bn_beta, bn_mean, bn_var):
    if "nc" not in _cache:
        _cache["nc"] = _build()
    nc = _cache["nc"]
    in_maps = host_inputs(feat, W_proj, b_proj, bn_gamma, bn_beta,
                          bn_mean, bn_var)
    res = run_bass_kernel_spmd(nc, in_maps, core_ids=list(range(N_CORES)))
    return np.concatenate([res.results[i]["out"] for i in range(N_CORES)],
                          axis=0)
